# revision 1
# baseline (speedup 1.0000x reference)
"""BP LDPC decoder as a sparse/segment Trainium2 kernel.

The reference multiplies dense [E,E] (E=3456) "exclusive sum" operators every
iteration.  Those operators are just the check-node / variable-node exclusive
sums of a (DV=3)-regular LDPC graph, so here the whole iteration is done with
per-edge elementwise ops + two indirect DMAs (scatter per-edge values into a
check-padded table, reduce, gather the per-check totals back per edge).  The
E x E / E x N matrices never touch the device.

Sharding: pure data parallel over the batch (16 -> 2 samples on each of the
8 cores); the graph structure (offset tables) is replicated.
"""

import sys
import numpy as np

for _p in ("/opt/trn_rl_repo", "/root/.axon_site/_ro/trn_rl_repo"):
    if _p not in sys.path:
        sys.path.insert(0, _p)

N = 1152          # variables
E = 3456          # edges (DV=3 per variable)
B = 16            # batch
NCORES = 8
BP = B // NCORES  # batch per core
NQ = N // 128     # variables per partition
M2 = 640          # padded check count (128 * 5)
S = M2 // 128     # checks per partition
DMAX = 16         # max check degree
RC = 2            # row channels: (lt, b)

_A32 = float(np.float32(1.0 + 1e-8))
_B32 = float(np.float32(1.0 - 1e-8))


def _derive_structure(H_sumC_to_V, H_xe_v_sumc_to_y):
    """Recover the LDPC graph from the dense operators.

    Device edge order is col-major (variable-major): e = 3*v + j, with
    variable v on partition v // NQ.  Returns int32 offset tables indexed by
    device edge order:
      offs_scatter[e] = row in the check-padded table T1 (= c*DMAX + k)
      offs_tot[e]     = row in the per-check totals table T2 (= c)
    """
    H_sumC_to_V = np.asarray(H_sumC_to_V)
    H_xe_v_sumc_to_y = np.asarray(H_xe_v_sumc_to_y)
    cols_rm = np.argmax(H_xe_v_sumc_to_y, axis=0)        # variable of row-major edge
    p_r2l = np.argsort(cols_rm, kind="stable")           # col-major pos -> row-major idx
    p_l2r = np.argsort(p_r2l)
    Br = H_sumC_to_V[:, p_l2r]                           # same-check adjacency (row-major)
    same = Br[np.arange(E - 1), np.arange(1, E)] > 0
    check_id = np.concatenate([[0], np.cumsum(~same)]).astype(np.int64)
    deg = np.bincount(check_id)
    assert deg.max() <= DMAX, f"check degree {deg.max()} > {DMAX}"
    assert check_id[-1] < M2
    start = np.concatenate([[0], np.cumsum(deg)[:-1]])
    k_slot = np.arange(E) - start[check_id]
    r = p_r2l                                            # row-major index of device edge e
    # device edge order: e = 3v+j = (p*9+q)*3+j -> [128, 27] (partition, slot)
    offs_scatter = (check_id[r] * DMAX + k_slot[r]).astype(np.int32).reshape(128, 27)
    offs_tot = check_id[r].astype(np.int32).reshape(128, 27)
    return offs_scatter, offs_tot


def _build_program(n_iters: int):
    import concourse.bacc as bacc
    import concourse.hw_specs as hw_specs
    # Force every activation onto the one table set containing all our
    # functions (ln/exp/sign/abs); otherwise the chooser ping-pongs between
    # sets and reloads tables 4x per iteration (~41us of ACT_TABLE_LOAD).
    if not getattr(bacc, "_act_tables_pinned", False):
        _orig_get = hw_specs.get_activation_tables

        def _pinned(arch):
            tabs = _orig_get(arch)
            keep = "natural_log_exp_and_others"
            if keep in tabs:
                tabs = {k: (v if k == keep else set()) for k, v in tabs.items()}
            return tabs

        bacc.get_activation_tables = _pinned
        bacc._act_tables_pinned = True
    import concourse.mybir as mybir
    import concourse.tile as tile
    from concourse.bass import IndirectOffsetOnAxis

    f32 = mybir.dt.float32
    i32 = mybir.dt.int32
    AF = mybir.ActivationFunctionType
    ALU = mybir.AluOpType

    nc = bacc.Bacc("TRN2", target_bir_lowering=False, debug=False)

    llr_d = nc.declare_dram_parameter("llr", [BP, N], f32, isOutput=False)
    osc_d = nc.declare_dram_parameter("offs_scatter", [128, 27], i32, isOutput=False)
    ott_d = nc.declare_dram_parameter("offs_tot", [128, 27], i32, isOutput=False)
    dec_d = nc.declare_dram_parameter("dec", [BP, N], i32, isOutput=True)

    T1 = nc.dram_tensor("T1", [M2 * DMAX, RC * BP], f32)   # check-padded per-edge rows
    T2 = nc.dram_tensor("T2", [M2, RC * BP], f32)          # per-check totals

    with tile.TileContext(nc) as tc:
        with tc.tile_pool(name="st", bufs=1) as st:
            llr_sb = st.tile([128, NQ, BP], f32)
            xe0 = st.tile([128, NQ, 3, BP], f32)
            x = st.tile([128, NQ, 3, BP], f32)
            offs_sc = st.tile([128, 27], i32)
            offs_tt = st.tile([128, 27], i32)
            pair = st.tile([128, NQ, 3, RC, BP], f32)      # (lt, b) per edge
            P = st.tile([128, S, DMAX, RC, BP], f32)       # check-padded gather-in
            tot = st.tile([128, S, RC, BP], f32)           # (Lt, K) per check
            G = st.tile([128, NQ, 3, RC, BP], f32)         # per-edge totals
            d = st.tile([128, NQ, 3, RC, BP], f32)         # (sr, kx) per edge
            ax = st.tile([128, NQ, 3, BP], f32)
            u = st.tile([128, NQ, 3, BP], f32)
            lnum = st.tile([128, NQ, 3, BP], f32)
            lden = st.tile([128, NQ, 3, BP], f32)
            xp = st.tile([128, NQ, 3, BP], f32)
            m = st.tile([128, NQ, 3, BP], f32)
            sgnx = st.tile([128, NQ, 3, BP], f32)
            w = st.tile([128, NQ, 3, BP], f32)
            pd = st.tile([128, NQ, 3, BP], f32)
            ki = st.tile([128, NQ, 3, BP], i32)
            kb = st.tile([128, NQ, 3, BP], i32)
            num = st.tile([128, NQ, 3, BP], f32)
            den = st.tile([128, NQ, 3, BP], f32)
            y = st.tile([128, NQ, 3, BP], f32)
            V = st.tile([128, NQ, BP], f32)
            bp_t = st.tile([128, NQ, BP], f32)
            sg = st.tile([128, NQ, BP], f32)
            dec_f = st.tile([128, NQ, BP], f32)
            dec_i = st.tile([128, NQ, BP], i32)
            zeros = st.tile([128, S * DMAX * RC * BP], f32)

            # ---- init ----
            nc.sync.dma_start(
                out=llr_sb[:], in_=llr_d.ap().rearrange("b (p q) -> p q b", p=128)
            )
            nc.sync.dma_start(out=offs_sc[:], in_=osc_d.ap())
            nc.sync.dma_start(out=offs_tt[:], in_=ott_d.ap())
            nc.vector.memset(zeros[:], 0.0)
            nc.sync.dma_start(
                out=T1.ap().rearrange("(p r) c -> p (r c)", p=128), in_=zeros[:]
            )
            bc = llr_sb[:].unsqueeze(2).to_broadcast([128, NQ, 3, BP])
            nc.vector.tensor_copy(xe0[:], bc)
            nc.vector.tensor_copy(x[:], xe0[:])

            pair_lt = pair[:, :, :, 0, :]
            pair_b = pair[:, :, :, 1, :]

            for t in range(n_iters):
                # lt = ln(1e-8 + tanh(|x|/2)) computed exp/ln-only:
                #   u = exp(-|x|); lt = ln(A - B*u) - ln(1 + u)
                nc.scalar.activation(ax[:], x[:], AF.Abs)
                nc.scalar.activation(u[:], ax[:], AF.Exp, scale=-1.0)
                nc.scalar.activation(lnum[:], u[:], AF.Ln, bias=_A32, scale=-_B32)
                nc.scalar.activation(lden[:], u[:], AF.Ln, bias=1.0)
                nc.vector.tensor_tensor(pair_lt, lnum[:], lden[:], ALU.subtract)
                # b = 1 if x < 0 else 0   (sign bookkeeping for the check product)
                nc.vector.tensor_scalar(pair_b, x[:], 0.0, None, ALU.is_lt)

                # scatter per-edge (lt, b) rows into the check-padded table
                # (verified primitive: one run per partition per DMA)
                pairw = pair[:].rearrange("p a b c d -> p (a b) (c d)")
                for sl in range(27):
                    nc.gpsimd.indirect_dma_start(
                        out=T1.ap(),
                        out_offset=IndirectOffsetOnAxis(ap=offs_sc[:, sl:sl + 1], axis=0),
                        in_=pairw[:, sl, :],
                        in_offset=None,
                    )
                # dense load back as [check, slot] and reduce over slots
                nc.sync.dma_start(
                    out=P[:].rearrange("p a b c d -> p (a b c d)"),
                    in_=T1.ap().rearrange("(p r) c -> p (r c)", p=128),
                )
                nc.vector.tensor_reduce(
                    tot[:], P[:].transpose([0, 1, 3, 4, 2]),
                    axis=mybir.AxisListType.X, op=ALU.add,
                )
                # totals out to T2, gather back per edge
                nc.sync.dma_start(
                    out=T2.ap().rearrange("(p s) c -> p (s c)", p=128), in_=tot[:]
                )
                Gw = G[:].rearrange("p a b c d -> p (a b) (c d)")
                for sl in range(27):
                    nc.gpsimd.indirect_dma_start(
                        out=Gw[:, sl, :],
                        out_offset=None,
                        in_=T2.ap(),
                        in_offset=IndirectOffsetOnAxis(ap=offs_tt[:, sl:sl + 1], axis=0),
                    )

                # exclusive check sums: sr = Lt - lt, kx = K - b (packed)
                nc.vector.tensor_tensor(d[:], G[:], pair[:], ALU.subtract)
                nc.scalar.activation(xp[:], d[:, :, :, 0, :], AF.Exp)
                # sign of exclusive product: (-1)^kx
                nc.vector.tensor_copy(ki[:], d[:, :, :, 1, :])
                nc.vector.tensor_scalar(kb[:], ki[:], 1, None, ALU.bitwise_and)
                nc.vector.tensor_scalar(sgnx[:], kb[:], -2.0, 1.0, ALU.mult, ALU.add)
                nc.vector.tensor_scalar(w[:], xp[:], -2e-7, None, ALU.add)
                nc.vector.tensor_tensor(pd[:], w[:], sgnx[:], ALU.mult)
                # y = ln((1 + pd) / (1 - pd + 1e-10))
                nc.scalar.activation(num[:], pd[:], AF.Ln, bias=1.0)
                nc.scalar.activation(den[:], pd[:], AF.Ln, bias=float(np.float32(1.0 + 1e-10)), scale=-1.0)
                nc.vector.tensor_tensor(y[:], num[:], den[:], ALU.subtract)

                # variable-node side is local: V = sum_j y
                nc.vector.tensor_reduce(
                    V[:], y[:].transpose([0, 1, 3, 2]),
                    axis=mybir.AxisListType.X, op=ALU.add,
                )
                if t < n_iters - 1:
                    nc.vector.tensor_tensor(w[:], xe0[:], y[:], ALU.subtract)
                    nc.vector.tensor_tensor(
                        x[:], w[:],
                        V[:].unsqueeze(2).to_broadcast([128, NQ, 3, BP]), ALU.add,
                    )
                else:
                    nc.vector.tensor_tensor(bp_t[:], llr_sb[:], V[:], ALU.add)
                    nc.scalar.activation(sg[:], bp_t[:], AF.Sign)
                    nc.vector.tensor_scalar(dec_f[:], sg[:], -0.5, 0.5, ALU.mult, ALU.add)
                    nc.vector.tensor_copy(dec_i[:], dec_f[:])
                    nc.sync.dma_start(
                        out=dec_d.ap().rearrange("b (p q) -> p q b", p=128),
                        in_=dec_i[:],
                    )
    nc.compile()
    return nc


_PROGRAM_CACHE = {}


def _get_program(n_iters: int):
    if n_iters not in _PROGRAM_CACHE:
        _PROGRAM_CACHE[n_iters] = _build_program(n_iters)
    return _PROGRAM_CACHE[n_iters]


def _make_in_maps(llr_in, H_sumC_to_V, H_xe_v_sumc_to_y):
    llr = np.ascontiguousarray(np.asarray(llr_in, dtype=np.float32))
    assert llr.shape == (B, N)
    offs_sc, offs_tt = _derive_structure(H_sumC_to_V, H_xe_v_sumc_to_y)
    osc = np.ascontiguousarray(offs_sc)
    ott = np.ascontiguousarray(offs_tt)
    return [
        {
            "llr": np.ascontiguousarray(llr[c * BP:(c + 1) * BP]),
            "offs_scatter": osc,
            "offs_tot": ott,
        }
        for c in range(NCORES)
    ]


def kernel(llr_in, H_x_to_xe0, H_sumC_to_V, H_sumV_to_C, H_xe_v_sumc_to_y,
           bp_iter_num, **_unused):
    from concourse.bass_utils import run_bass_kernel_spmd

    n_iters = int(np.asarray(bp_iter_num))
    nc = _get_program(n_iters)
    in_maps = _make_in_maps(llr_in, H_sumC_to_V, H_xe_v_sumc_to_y)
    res = run_bass_kernel_spmd(nc, in_maps, list(range(NCORES)))
    out = np.concatenate([res.results[c]["dec"] for c in range(NCORES)], axis=0)
    return out.astype(np.int32)

